# revision 1
# baseline (speedup 1.0000x reference)
"""Batch-parallel dot-product attention for Trainium2 (Bass/Tile).

Problem: B=8, Q=K=2048, D=128, fp32, with a [B, K] 0/1 attention mask.
Sharding: one batch element per NeuronCore (8 cores), no collectives.

The mask is per-key and typically zeroes ~half the keys. The host computes
per-batch kept-key indices (order is irrelevant to softmax) and an additive
bias vector; the device gathers kept [K | V] rows (host-concatenated, 1KB
each) with indirect DMAs and runs attention over the compacted context,
padded to a multiple of 128 and shared across cores. Consecutive kept-key
pairs are packed into the leading k-tiles so one gather fills two tiles
(the hardware fetches consecutive rows for a flat [P, 2*rowlen] out AP).
A dense variant builds automatically when compaction wouldn't shrink the
context; an all-masked batch degenerates to the reference's uniform
softmax via an all-zero bias.

Per-core pipeline (every tensor uses a "(p t)" index split so all large
DMAs are contiguous per partition; the split is applied consistently to
q, k/v, bias, indices and the output, so results are exact):

  1. Q arrives pre-transposed from the host in the [d, (t p)] layout and
     DMAs straight into a float32r tile (walrus accepts a DMA producer).
     K^T is built on-device: PE transposes in 2-tile flushes staged through
     a dedicated 1-bank PSUM tag. Dummy fp16 matmuls warm the PE's HAM
     clock gate during the initial DMA wait.
  2. Phase A (per k-tile): S^T[k, q] = (K^T_tile).T @ Q^T via float32r
     matmuls (full PE rate at moving dim 512) into double-buffered
     [128, 1024] PSUM score slots, one query-half at a time.
  3. Masked exp on ScalarE straight out of PSUM:
     E = exp(S_raw / sqrt(D) + bias_k), bias 0 (kept) or -1e6 (masked or
     padding), per partition since k sits on partitions in S^T; fp16 out.
     The exp stream runs gap-free and is the bound engine.
  4. Phase B (per 128-query sub-block): out[q, 0:129] = sum_kt E_kt.T @
     [V|1] accumulated in PSUM; the ones column makes the softmax
     denominator a free by-product. Each accumulator owns a full PSUM bank
     (matmul start=True zeroes the whole 2KB zero-region). Single-subblock
     waves rotate their k-tile order and are software-pipelined against
     phase A through a deferred-chunk FIFO (drained before the A matmuls
     and after the exp of each slot), with three accumulator banks.
  5. Normalize out = out[:, :128] * reciprocal(out[:, 128]) on VectorE;
     stores go out two sub-blocks per DMA, gated by an order-independent
     completion tracker (Tile tracks dependencies by emission order).

PSUM budget (8 banks): 2x2 score slots + 1 transpose-flush bank + 3
phase-B accumulator banks.
"""

import math
from contextlib import ExitStack

import numpy as np

import concourse.bass as bass
import concourse.mybir as mybir
import concourse.tile as tile
from concourse import bacc
from concourse.bass import ds, ts
B = 8
SEQ = 2048
D = 128
P = 128

F32 = mybir.dt.float32
F32R = mybir.dt.float32r
F16 = mybir.dt.float16
I32 = mybir.dt.int32

NEG_BIAS = -1.0e6  # matches the reference mask fill; exp() underflows to 0.0


def attention_kernel(tc, qt, k, v, kv, bias, ctli, o, seq, nctx, npair, compact):
    """compact=False => dense (nctx == seq, K/V loaded from k/v directly);
    otherwise K/V rows come from one gather per k-tile over the
    host-concatenated kv = [K | V] tensor (halving the per-gather fixed
    SWDGE cost). qt is Q pre-transposed on the host into the on-chip
    [d, (t p)] layout (value-independent), which deletes the whole
    Q-staging/transpose pipeline; K^T still must be transposed on-device
    because its rows come from the runtime gather."""
    nc = tc.nc
    nkt = nctx // P         # context k-tiles
    qh = 2                  # query halves (PSUM capacity forces 2 passes)
    qc = seq // qh          # queries per half
    nqs = qc // P           # 128-query sub-blocks per half
    scale = 1.0 / math.sqrt(D)
    exp_f = mybir.ActivationFunctionType.Exp
    with ExitStack() as ctx:
        constp = ctx.enter_context(tc.tile_pool(name="constp", bufs=1))
        stagep = ctx.enter_context(tc.tile_pool(name="stagep", bufs=1))
        sqp = ctx.enter_context(tc.tile_pool(name="sqp", bufs=1))
        ep = ctx.enter_context(tc.tile_pool(name="ep", bufs=2))
        smallp = ctx.enter_context(tc.tile_pool(name="smallp", bufs=4))
        psumA = ctx.enter_context(tc.tile_pool(name="psumA", bufs=2, space="PSUM"))
        psumB = ctx.enter_context(tc.tile_pool(name="psumB", bufs=3, space="PSUM"))


        if compact:
            kvst = stagep.tile([P, nkt, 2 * D], F32, tag="kvstage", name="kvst")
            kst = kvst[:, :, 0:D]
            vst = kvst[:, :, D : 2 * D]
        else:
            kst = stagep.tile([P, nkt, D], F32, tag="kstage", name="kst")
            vst = stagep.tile([P, nkt, D], F32, tag="vstage", name="vst")

        # Startup latency is dominated by per-DMA issue+completion (~1.5us
        # to land each sync-queue DMA), so: (1) the kept-key indices and the
        # transpose identity ship together as ONE leading DMA, (2) the first
        # q chunks ride the Activation engine's separate HWDGE ring, (3) the
        # bias goes second on sync (it gates only the first exp).
        ctli_sb = constp.tile([P, nkt + P], F32)
        nc.sync.dma_start(ctli_sb, ctli)
        ixt = ctli_sb[:, 0:nkt].bitcast(I32) if compact else None
        ident = ctli_sb[:, nkt : nkt + P]

        def ix_col(t):
            return ixt[:, t : t + 1]

        # per-key additive bias (0 kept / -1e6 masked or padding)
        bv = constp.tile([P, nkt], F32)
        nc.sync.dma_start(bv, bias.rearrange("(p t) -> p t", p=P))

        # Dummy exp early so walrus front-loads the ACT table load under the
        # input DMAs instead of serializing it before the first real exp.
        warm = smallp.tile([P, 1], F32, tag="warm")
        nc.vector.memset(warm, 0.0)
        nc.scalar.activation(warm, warm, exp_f)

        # The PE sits idle for the first ~4us waiting on input DMAs, which
        # leaves the HAM clock gate cold (1.2 GHz) exactly when the first
        # transposes and matmuls run. Dummy fp16 matmuls on a zeroed tile
        # keep the PE busy through the wait so the real work starts at the
        # full 2.4 GHz rate.
        wm16 = smallp.tile([P, P], F16, tag="wm16")
        nc.vector.memset(wm16, 0.0)
        pwarm = psumA.tile([P, 2 * P], F32, tag="sA", name="pwarm")
        for _ in range(18):
            nc.tensor.matmul(
                pwarm[:, 0:P], lhsT=wm16, rhs=wm16, start=True, stop=True
            )


        # V as fp16 with a ones column appended: rhs of phase B. The ones
        # column has no data dependency, the value columns are cast as the
        # corresponding V tiles land.
        vp = constp.tile([P, nkt, D + 4], F16)
        nc.vector.memset(vp[:, :, D : D + 1], 1.0)

        # K/V: indirect row gathers (compact) or straight loads (dense), on
        # the gpsimd queue so they overlap the sync-queue q loads.
        if compact:
            # The hardware gather honors exactly ONE offset per partition and
            # a flat 2D out AP; extra out columns receive CONSECUTIVE source
            # rows (verified on HW). The host packs consecutive kept-key
            # pairs into the first 2*npair k-tiles, so each pair gather
            # fetches TWO tiles of [K | V] rows with one instruction; the
            # remaining keys use one single-row gather per tile.
            kvf = kvst.rearrange("p t d -> p (t d)")
            for g in range(npair):
                nc.gpsimd.indirect_dma_start(
                    out=kvf[:, ds(g * 4 * D, 4 * D)],
                    out_offset=None,
                    in_=kv[:],
                    in_offset=bass.IndirectOffsetOnAxis(ap=ix_col(2 * g), axis=0),
                )
            for t in range(2 * npair, nkt):
                nc.gpsimd.indirect_dma_start(
                    out=kvst[:, t, :],
                    out_offset=None,
                    in_=kv[:],
                    in_offset=bass.IndirectOffsetOnAxis(ap=ix_col(t), axis=0),
                )
        else:
            k_re = k.rearrange("(p t) d -> p t d", p=P)
            nc.gpsimd.dma_start(kst[:, 0:4], k_re[:, 0:4])
            if nkt > 4:
                nc.gpsimd.dma_start(kst[:, 4:nkt], k_re[:, 4:nkt])
            nc.gpsimd.dma_start(vst, v.rearrange("(p t) d -> p t d", p=P))

        # Q^T / K^T: [128 d, n] with the (p t) scramble on the free axis.
        # Q^T loads directly (host supplies the transposed layout) on the
        # ScalarE HWDGE ring, in halves so phase A starts after the first.
        qT = sqp.tile([P, seq], F32R, tag="qT")
        kT = sqp.tile([P, nctx], F32R, tag="kT")
        nc.scalar.dma_start(qT[:, 0 : seq // 2], qt[:, 0 : seq // 2])
        nc.scalar.dma_start(qT[:, seq // 2 :], qt[:, seq // 2 :])

        def emit_flush(st, dstT, f, ntile):
            # PE transposes in flushes of <=2 tiles; short psumA-slot
            # residency keeps phase A's double-buffering alive. The last k
            # flush may cover a single tile (odd k-tile counts).
            w = min(2, ntile - 2 * f)
            # dedicated 1-bank tag: flushes must never steal a score slot
            # (that breaks the exp double-buffer and stalls ScalarE)
            pt = psumA.tile([P, 2 * P], F32, tag="fl", bufs=1, name=f"pt_{f}")
            for j in range(w):
                nc.tensor.transpose(pt[:, ts(j, P)], st[:, 2 * f + j], ident)
            nc.vector.tensor_copy(dstT[:, ds(2 * P * f, w * P)], pt[:, 0 : w * P])

        k_fl_total = (nkt + 1) // 2
        emit_flush(kst, kT, 0, nkt)
        k_done = 1

        # V -> fp16 casts, lazily: tile t is cast once its gather has had
        # time to land, so the DVE never stalls with flush copies queued
        # behind it. Dense mode casts everything at once.
        vcast_state = {"done": 0}

        def emit_vcast(upto):
            upto = min(upto, nkt)
            if vcast_state["done"] >= upto:
                return
            nc.vector.tensor_copy(
                vp[:, vcast_state["done"] : upto, 0:D],
                vst[:, vcast_state["done"] : upto],
            )
            vcast_state["done"] = upto

        if not compact:
            emit_vcast(nkt)
        else:
            emit_vcast(1)

        # Full-size output buffer (fp32), stored contiguously at half bounds
        outbuf = constp.tile([P, seq // P, D], F32)

        deferred = []  # (min_slot, emit_fn) FIFO of phase-B chunks
        norm_done = set()  # normalized query-subblocks (store pairing)

        o_re = o.rearrange("(p t) d -> p t d", p=P)

        def make_wave(h, et_h, qs_list, rot=0):
            # Accumulation order over k-tiles is free, so each wave processes
            # them rotated by `rot`: staggered waves become eligible as soon
            # as ACT finishes their own first k-tile, instead of all waves
            # queueing on the half's LAST k-tile.
            state = {}

            def chunk(i):
                kt = (rot + i) % nkt
                if i == 0:
                    state["oacc"] = {
                        qs: psumB.tile(
                            [P, 132], F32, tag="oacc", name=f"oacc_{h}_{qs}"
                        )
                        for qs in qs_list
                    }
                for qs in qs_list:
                    nc.tensor.matmul(
                        state["oacc"][qs][:, 0 : D + 1],
                        lhsT=et_h[:, kt, ts(qs, P)],
                        rhs=vp[:, kt, 0 : D + 1],
                        start=(i == 0),
                        stop=(i == nkt - 1),
                    )
                if i == nkt - 1:
                    for qs in qs_list:
                        tg = h * nqs + qs
                        r = smallp.tile([P, 1], F32, tag="r")
                        nc.vector.reciprocal(r, state["oacc"][qs][:, D : D + 1])
                        nc.vector.tensor_scalar_mul(
                            outbuf[:, tg, :], state["oacc"][qs][:, 0:D], r
                        )
                    # batch stores two query-subblocks per DMA (per-wave
                    # stores at wsz=1 would pay ~650ns of sync-queue issue
                    # each). Emission must follow BOTH normalizes (Tile
                    # tracks dependencies by emission order), so a shared
                    # tracker fires the store when its pair completes.
                    for qs in qs_list:
                        tg = h * nqs + qs
                        norm_done.add(tg)
                        lo = tg - (tg % 2)
                        hi = min(lo + 1, qh * nqs - 1)
                        if all(x in norm_done for x in range(lo, hi + 1)):
                            nc.sync.dma_start(
                                o_re[:, lo : hi + 1], outbuf[:, lo : hi + 1]
                            )

            return chunk

        wsz = 1  # wave size (PSUM banks per wave)
        for h in range(qh):
            bq = h * qc
            et = ep.tile([P, nkt, qc], F16, tag="et")
            wave0 = make_wave(h, et, list(range(min(wsz, nqs))))
            for wj, w0 in enumerate(range(wsz, nqs, wsz)):
                qs_list = list(range(w0, min(w0 + wsz, nqs)))
                # first two deferred waves hold banks through the loop:
                # small rotations minimize their post-exp remainder; later
                # waves burst post-exp regardless
                rot = (1 + wj) % nkt if wj < 2 else (2 + 3 * wj) % nkt
                wv = make_wave(h, et, qs_list, rot=rot)
                for i in range(nkt):
                    ms = (rot + i) % nkt + 1
                    deferred.append((ms, lambda wv=wv, i=i: wv(i)))

            for kt in range(nkt):
                # drain some eligible deferred phase-B work first: if phase A
                # is about to stall on a gather/transpose, the PE chews useful
                # B matmuls instead of idling in-order behind it
                popped = 0
                while deferred and popped < 2 and deferred[0][0] <= kt:
                    deferred.pop(0)[1]()
                    popped += 1
                if h == 0:
                    # K^T flush needed by this k-tile
                    while k_done * 2 <= kt + 1 and k_done < k_fl_total:
                        emit_flush(kst, kT, k_done, nkt)
                        k_done += 1
                if h == 0:
                    emit_vcast(kt + 3)
                pa = psumA.tile([P, qc], F32, tag="sA")
                lk = kT[:, ts(kt, P)]
                chunk = min(512, qc)
                for c in range(qc // chunk):
                    nc.tensor.matmul(
                        pa[:, ts(c, chunk)],
                        lhsT=lk,
                        rhs=qT[:, ds(bq + c * chunk, chunk)],
                        start=True,
                        stop=True,
                    )
                nc.scalar.activation(
                    et[:, kt, :], pa, exp_f, bias=bv[:, kt : kt + 1], scale=scale
                )
                if kt > 0:
                    wave0(kt - 1)
                # post-ACT drain: chunks for THIS slot's k-tile are now safe
                popped = 0
                while deferred and popped < 4 and deferred[0][0] <= kt + 1:
                    deferred.pop(0)[1]()
                    popped += 1
            wave0(nkt - 1)
            # leftover K^T flushes (short-context edge cases)
            if h == 0:
                while k_done < k_fl_total:
                    emit_flush(kst, kT, k_done, nkt)
                    k_done += 1
            # anything left is fully unblocked once this half's ACTs are done
            deferred[:] = [(0, fn) for _, fn in deferred]

        while deferred:
            deferred.pop(0)[1]()


def build_nc(seq=SEQ, nctx=None, npair=0, n_cores=B):
    compact = nctx is not None and nctx < seq
    nc = bacc.Bacc(
        "TRN2", target_bir_lowering=False, debug=False, num_devices=n_cores
    )
    qt = nc.dram_tensor("qt", [D, seq], F32R, kind="ExternalInput").ap()
    if compact:
        k = v = None
        kv = nc.dram_tensor("kv", [seq, 2 * D], F32, kind="ExternalInput").ap()
    else:
        k = nc.dram_tensor("k", [seq, D], F32, kind="ExternalInput").ap()
        v = nc.dram_tensor("v", [seq, D], F32, kind="ExternalInput").ap()
        kv = None
    bias = nc.dram_tensor("bias", [nctx], F32, kind="ExternalInput").ap()
    nkt = nctx // P
    ctli = nc.dram_tensor("ctli", [P, nkt + P], F32, kind="ExternalInput").ap()
    o = nc.dram_tensor("o", [seq, D], F32, kind="ExternalOutput").ap()
    with nc.allow_low_precision("softmax reciprocal on VectorE"):
        with tile.TileContext(nc) as tc:
            attention_kernel(
                tc, qt, k, v, kv, bias, ctli, o, seq, nctx, npair, compact
            )
    nc.compile()
    return nc


_NC_CACHE = {}


def _get_nc(seq, nctx, npair):
    key = (seq, nctx, npair)
    if key not in _NC_CACHE:
        _NC_CACHE[key] = build_nc(seq=seq, nctx=nctx, npair=npair)
    return _NC_CACHE[key]


def _greedy_pairs(kept):
    """Disjoint consecutive (r, r+1) pairs among kept rows + leftovers."""
    pairs, singles = [], []
    i, L = 0, len(kept)
    while i < L:
        if i + 1 < L and kept[i + 1] == kept[i] + 1:
            pairs.append(int(kept[i]))
            i += 2
        else:
            singles.append(int(kept[i]))
            i += 1
    return np.asarray(pairs, np.int32), np.asarray(singles, np.int32)


def prepare(queries, keys, values, attntion_mask):
    """Host-side: per-batch kept-key indices + bias, padded context size."""
    nb = queries.shape[0]
    seq = queries.shape[1]
    kept = [np.flatnonzero(attntion_mask[b]).astype(np.int32) for b in range(nb)]
    n_max = max(int(kk.size) for kk in kept)
    nctx = min(seq, max(128, ((max(n_max, 1) + 127) // 128) * 128))
    nkt = nctx // P
    # Pack consecutive kept pairs into the leading 2*npair k-tiles: one
    # gather instruction fetches TWO tiles there (HW fetches consecutive
    # rows). npair is shared across batches (one NEFF) and only as large as
    # still fits everything in the same nctx.
    pr = [_greedy_pairs(kk) for kk in kept]
    npair = min(len(p) for p, s in pr) // P if nctx < seq else 0
    while npair > 0:
        rem_tiles = max(
            -(-(int(kk.size) - 2 * P * npair) // P) for kk in kept
        )
        if 2 * npair + max(rem_tiles, 0) <= nkt:
            break
        npair -= 1
    in_maps = []
    eye = np.eye(P, dtype=np.float32)
    tpq = seq // P
    for b in range(nb):
        n = int(kept[b].size)
        # Q pre-transposed into the scrambled on-chip layout:
        # qt[d, t*P + p] = Q[p*tpq + t, d]
        m = {
            "qt": np.ascontiguousarray(
                queries[b]
                .reshape(P, tpq, D)
                .transpose(2, 1, 0)
                .reshape(D, seq),
                dtype=np.float32,
            )
        }
        bias = np.full(nctx, NEG_BIAS, dtype=np.float32)
        idx = np.zeros(nctx, dtype=np.int32)
        if nctx < seq:
            m["kv"] = np.ascontiguousarray(
                np.concatenate([keys[b], values[b]], axis=1), dtype=np.float32
            )
            pairs_b, singles_b = pr[b]
            ix2 = idx.reshape(P, nkt)
            bv2 = bias.reshape(P, nkt)
            for g in range(npair):
                arr = pairs_b[g * P : (g + 1) * P]
                ix2[:, 2 * g] = arr
                ix2[:, 2 * g + 1] = arr + 1
                bv2[:, 2 * g : 2 * g + 2] = 0.0
            lo = pairs_b[npair * P :]
            rest = np.concatenate([singles_b, lo, lo + 1]).astype(np.int32)
            ns = nkt - 2 * npair
            tmp_i = np.zeros(P * ns, np.int32)
            tmp_b = np.full(P * ns, NEG_BIAS, np.float32)
            tmp_i[: rest.size] = rest
            tmp_b[: rest.size] = 0.0
            ix2[:, 2 * npair :] = tmp_i.reshape(P, ns)
            bv2[:, 2 * npair :] = tmp_b.reshape(P, ns)
        else:
            m["k"] = np.ascontiguousarray(keys[b], dtype=np.float32)
            m["v"] = np.ascontiguousarray(values[b], dtype=np.float32)
            # dense fallback (also covers the all-masked batch, which the
            # reference treats as a uniform softmax over every key)
            if n == 0:
                bias[:] = 0.0
            else:
                bias[:seq] = np.where(
                    attntion_mask[b] != 0, 0.0, NEG_BIAS
                ).astype(np.float32)
        m["bias"] = bias
        # [idx bits | identity] in the (p t) layout, one leading DMA
        m["ctli"] = np.ascontiguousarray(
            np.concatenate([idx.reshape(P, nkt).view(np.float32), eye], axis=1),
            dtype=np.float32,
        )
        in_maps.append(m)
    return nctx, npair, in_maps


def kernel(queries, keys, values, attntion_mask, **run_kwargs):
    from concourse.bass_utils import run_bass_kernel_spmd

    queries = np.asarray(queries)
    keys = np.asarray(keys)
    values = np.asarray(values)
    attntion_mask = np.asarray(attntion_mask)
    nctx, npair, in_maps = prepare(queries, keys, values, attntion_mask)
    nc = _get_nc(queries.shape[1], nctx, npair)
    res = run_bass_kernel_spmd(
        nc,
        in_maps,
        core_ids=list(range(queries.shape[0])),
        **run_kwargs,
    )
    out = np.stack([r["o"] for r in res.results], axis=0).astype(np.float32)
    if run_kwargs:
        kernel.last_results = res
    return out



# revision 25
# speedup vs baseline: 1.2688x; 1.2688x over previous
"""Batch-parallel dot-product attention for Trainium2 (Bass/Tile), v2.

Problem: B=8, Q=K=2048, D=128, fp32, with a [B, K] 0/1 attention mask.
Sharding: one batch element per NeuronCore (8 cores), no collectives.

Architecture (query-subblock-major):

The host compacts the context per batch: kept keys (mask=1) are gathered
on the host into a padded [nctx, D] context (nctx = roundup(max kept)),
with zero rows as padding.  K^T ships pre-transposed in bf16; V ships as
fp16 rows with the key-validity mask appended as a 129th column
([V | mk]); Q^T ships in bf16.  All mask handling is data: padding keys
have zero V rows and mk=0, so they contribute exactly nothing to the
numerator (E x 0) or denominator (E x mk sums), and no bias add or
on-device gather/transpose is needed at all.  An all-masked batch is the
reference's uniform softmax; the host sends qt=0 and mk=1 for all keys,
making E identically 1 (exact).

Device, per 128-query subblock qs (16 of them):
  1. Phase A: 9 matmuls S[k_tile, q128] = kT_tile.T @ qT_qs (bf16, N=128)
     into one 3-bank PSUM slot laid out [128, nkt, 128].
  2. One exp over the whole slot (FD = nkt*128 = 1152) on ScalarE,
     PSUM -> SBUF fp16, scale=1/sqrt(D) folded in.  ScalarE is the
     bottleneck engine; 16 of these run back to back.
  3. Phase B: 9 matmuls accumulating O[q128, 129] = sum_t E_t.T @ [V|mk]_t
     in a 1-bank PSUM accumulator; the mk column makes the softmax
     denominator a free by-product (and implements the mask).
  4. Normalize on VectorE (reciprocal + per-partition scalar multiply)
     and store the 128 output rows per subblock on the sync queue.

Two PSUM score slots (3 banks each) + two accumulators (1 bank each)
fill all 8 banks and give a 2-deep software pipeline: A'(qs+2) and
B(qs) run on the PE while exp(qs+1) runs on ScalarE, keeping the exp
stream gap-free.  Dummy fp16 matmuls warm the PE clock gate and a dummy
exp front-loads the ACT table load during the input DMA wait.
"""

import math
from contextlib import ExitStack

import numpy as np

import concourse.bass as bass
import concourse.mybir as mybir
import concourse.tile as tile
from concourse import bacc
from concourse.bass import ds, ts

B = 8
SEQ = 2048
D = 128
P = 128
VROW = 132  # fp16 row: [V (128) | mk (1) | pad (3)]
HDR0 = 128  # leading Q^T columns packed into the hdr tensor

F32 = mybir.dt.float32
BF16 = mybir.dt.bfloat16
F16 = mybir.dt.float16

NWARM = 24  # PE clock-gate warm matmuls before the first real work
NWARM_FINE = 10  # small trailing warm matmuls (fine-grained busy-keeping)


def attention_kernel(tc, qt, hdr, vp, o, seq, nctx):
    nc = tc.nc
    nkt = nctx // P
    nqs = seq // P
    scale = 1.0 / math.sqrt(D)
    exp_f = mybir.ActivationFunctionType.Exp

    # k-tile groups per exp: one PSUM slot is <= 3 banks (12 k-tiles)
    ngr = -(-nkt // 12)
    gbase, grem = divmod(nkt, ngr)
    sizes = [gbase + (1 if i < grem else 0) for i in range(ngr)]
    starts = [sum(sizes[:i]) for i in range(ngr)]
    slot_bufs = 2 if ngr == 1 else 1
    units = [(qs, g) for qs in range(nqs) for g in range(ngr)]

    with ExitStack() as ctx:
        sb = ctx.enter_context(tc.tile_pool(name="sb", bufs=1))
        # 3 et buffers so exp(i+2) never waits on B(i) to free its buffer
        etp = ctx.enter_context(tc.tile_pool(name="etp", bufs=3))
        obp = ctx.enter_context(tc.tile_pool(name="obp", bufs=3))
        smallp = ctx.enter_context(tc.tile_pool(name="smallp", bufs=4))
        psS = ctx.enter_context(tc.tile_pool(name="psS", bufs=slot_bufs, space="PSUM"))
        psO = ctx.enter_context(tc.tile_pool(name="psO", bufs=2, space="PSUM"))

        # DMA transfers serialize on the (aggregate-bandwidth) DMA engine
        # track, so issue order is priority order: a small leading Q^T chunk
        # (first two subblocks), then the whole of K^T (phase A of qs0 needs
        # every k-tile), then [V|mk] (needed by B(0)), then the rest of Q^T.
        # HWDGE descriptor generation (625ns/DMA) is a serial track shared by
        # the sync and scalar queues, and transfers serialize on the DMA
        # engines — so the critical path wants ONE leading DMA carrying
        # everything exp(0) needs: hdr = [Q^T cols 0:256 | K^T] (host-packed).
        hdrt = sb.tile([P, HDR0 + nctx], BF16)
        nc.sync.dma_start(hdrt, hdr)
        qc0 = hdrt[:, 0:HDR0]
        ktt = hdrt[:, HDR0 : HDR0 + nctx]
        # scalar ring: mid Q^T (gates A'(2..9)), then [V|mk] (first needed by
        # B(0), well after exp(0)), then the tail of Q^T.
        qtt = sb.tile([P, seq], BF16)
        vpt = sb.tile([P, nkt, VROW], F16)
        c1 = min(seq, HDR0 + 8 * P)
        nc.scalar.dma_start(qtt[:, HDR0:c1], qt[:, HDR0:c1])
        nc.scalar.dma_start(vpt.rearrange("p t d -> p (t d)"), vp)
        if c1 < seq:
            nc.scalar.dma_start(qtt[:, c1:seq], qt[:, c1:seq])

        # Dummy exp so walrus front-loads the ACT table load under the DMAs.
        # Memsets go to the otherwise-idle GpSimd engine so the PE warm
        # matmuls (below) start as early as possible — the clock-gate ramp
        # needs 3us of continuous PE busy before full rate.
        warm = smallp.tile([P, 1], F32, tag="warm")
        nc.gpsimd.memset(warm, 0.0)
        nc.scalar.activation(warm, warm, exp_f)

        # PE clock-gate warm: keep the PE busy through the input-DMA wait so
        # the real matmuls start at full rate.
        wm = smallp.tile([P, P], F16, tag="wm")
        nc.gpsimd.memset(wm, 0.0)
        pw = psO.tile([P, VROW], F32, tag="oacc", name="pw")
        for _ in range(NWARM):
            nc.tensor.matmul(pw[:, 0:P], lhsT=wm, rhs=wm, start=True, stop=True)
        # fine-grained warm tail (N=32) so the PE stays busy until the hdr
        # DMA lands without overshooting into A'(0)'s start
        for _ in range(NWARM_FINE):
            nc.tensor.matmul(
                pw[:, 0:32], lhsT=wm, rhs=wm[:, 0:32], start=True, stop=True
            )

        slots = {}
        ets = {}
        oaccs = {}

        def emit_A(u):
            qs, g = u
            sl = psS.tile(
                [P, sizes[g], P], F32, tag=f"sl{g}", bufs=slot_bufs,
                name=f"sl_{qs}_{g}",
            )
            slots[u] = sl
            for j in range(sizes[g]):
                t = starts[g] + j
                rhs = qc0 if qs * P < HDR0 else qtt
                nc.tensor.matmul(
                    sl[:, j, :], lhsT=ktt[:, ts(t, P)], rhs=rhs[:, ts(qs, P)],
                    start=True, stop=True,
                )

        def emit_exp(u, split_tail=0):
            qs, g = u
            if g == 0:
                ets[qs] = etp.tile([P, nkt, P], F16, tag="et", name=f"et{qs}")
            t0, sz = starts[g], sizes[g]
            sl = slots.pop(u)
            if 0 < split_tail < sz:
                # first unit: two activations so exp starts as soon as the
                # leading A' matmuls land (pipeline fill); final unit: the
                # trailing B/normalize/store chain hangs off a short exp.
                sa = sz - split_tail
                nc.scalar.activation(
                    ets[qs][:, t0 : t0 + sa, :], sl[:, 0:sa, :], exp_f,
                    scale=scale,
                )
                nc.scalar.activation(
                    ets[qs][:, t0 + sa : t0 + sz, :], sl[:, sa:sz, :], exp_f,
                    scale=scale,
                )
            else:
                nc.scalar.activation(
                    ets[qs][:, t0 : t0 + sz, :], sl, exp_f, scale=scale
                )

        def emit_B(u):
            qs, g = u
            if g == 0:
                oaccs[qs] = psO.tile([P, VROW], F32, tag="oacc", name=f"oacc{qs}")
            oa = oaccs[qs]
            et = ets[qs]
            for j in range(sizes[g]):
                t = starts[g] + j
                nc.tensor.matmul(
                    oa[:, 0 : D + 1], lhsT=et[:, t, :], rhs=vpt[:, t, 0 : D + 1],
                    start=(t == 0), stop=(t == nkt - 1),
                )

        def emit_norm_store(qs):
            oa = oaccs.pop(qs)
            r = smallp.tile([P, 1], F32, tag="r")
            nc.vector.reciprocal(r, oa[:, D : D + 1])
            ob = obp.tile([P, P], F32, tag="ob", name=f"ob{qs}")
            nc.vector.tensor_scalar_mul(ob, oa[:, 0:D], r)
            nc.sync.dma_start(o[ds(qs * P, P), :], ob)

        emit_A(units[0])
        if len(units) > 1:
            emit_A(units[1])
        for i, u in enumerate(units):
            emit_exp(u)
            if i + 2 < len(units):
                emit_A(units[i + 2])
            emit_B(u)
            if u[1] == ngr - 1:
                emit_norm_store(u[0])


def build_nc(seq=SEQ, nctx=SEQ, n_cores=B):
    nkt = nctx // P
    nc = bacc.Bacc(
        "TRN2", target_bir_lowering=False, debug=False, num_devices=n_cores
    )
    qt = nc.dram_tensor("qt", [D, seq], BF16, kind="ExternalInput").ap()
    hdr = nc.dram_tensor("hdr", [D, HDR0 + nctx], BF16, kind="ExternalInput").ap()
    vp = nc.dram_tensor("vp", [P, nkt * VROW], F16, kind="ExternalInput").ap()
    o = nc.dram_tensor("o", [seq, D], F32, kind="ExternalOutput").ap()
    with nc.allow_low_precision("softmax reciprocal on VectorE"):
        with tile.TileContext(nc) as tc:
            attention_kernel(tc, qt, hdr, vp, o, seq, nctx)
    nc.compile()
    return nc


_NC_CACHE = {}


def _get_nc(seq, nctx):
    key = (seq, nctx)
    if key not in _NC_CACHE:
        _NC_CACHE[key] = build_nc(seq=seq, nctx=nctx)
    return _NC_CACHE[key]


def prepare(queries, keys, values, attntion_mask):
    """Host-side layout prep: per-batch compacted context in low precision.

    Returns (nctx, in_maps)."""
    import ml_dtypes

    bf = ml_dtypes.bfloat16
    nb, seq, d = queries.shape
    masks = np.asarray(attntion_mask) != 0
    kept = [np.flatnonzero(masks[b]) for b in range(nb)]
    ns = [int(k.size) for k in kept]
    if min(ns) == 0:
        nctx = seq
    else:
        nctx = min(seq, ((max(ns) + P - 1) // P) * P)
    nkt = nctx // P
    in_maps = []
    for b in range(nb):
        n = ns[b]
        if n == 0:
            # all-masked: reference degenerates to a uniform softmax over
            # every key; qt=0 makes E identically 1, which is exact.
            idx = np.arange(nctx)
            mk = np.ones(nctx, np.float32)
            qtb = np.zeros((d, seq), np.float32)
        else:
            idx = np.zeros(nctx, np.int64)
            idx[:n] = kept[b]
            mk = np.zeros(nctx, np.float32)
            mk[:n] = 1.0
            qtb = queries[b].T
        kc = keys[b][idx] * mk[:, None]
        vc = values[b][idx] * mk[:, None]
        vpa = np.zeros((P, nkt, VROW), np.float16)
        vpa[:, :, 0:d] = vc.reshape(nkt, P, d).transpose(1, 0, 2)
        vpa[:, :, d] = mk.reshape(nkt, P).T
        qtb16 = np.ascontiguousarray(qtb).astype(bf)
        in_maps.append({
            "qt": qtb16,
            "hdr": np.ascontiguousarray(
                np.concatenate([qtb16[:, 0:HDR0], kc.T.astype(bf)], axis=1)
            ),
            "vp": np.ascontiguousarray(vpa.reshape(P, nkt * VROW)),
        })
    return nctx, in_maps


def kernel(queries, keys, values, attntion_mask, **run_kwargs):
    from concourse.bass_utils import run_bass_kernel_spmd

    queries = np.asarray(queries)
    keys = np.asarray(keys)
    values = np.asarray(values)
    attntion_mask = np.asarray(attntion_mask)
    nctx, in_maps = prepare(queries, keys, values, attntion_mask)
    nc = _get_nc(queries.shape[1], nctx)
    res = run_bass_kernel_spmd(
        nc,
        in_maps,
        core_ids=list(range(queries.shape[0])),
        **run_kwargs,
    )
    out = np.stack([r["o"] for r in res.results], axis=0).astype(np.float32)
    if run_kwargs:
        kernel.last_results = res
    return out
